# revision 46
# baseline (speedup 1.0000x reference)
"""MLA (multi-head latent attention) Trainium2 kernel.

Sharding: 8 cores = 2 (batch) x 4 (head groups of 4 heads).
Each core computes, for its batch b and heads [4g, 4g+4):
  latents kv_d/q_d (replicated within the batch group), per-head
  up-projections + RoPE, causal SDPA, and a partial o_proj
  out_core[o, q] = sum_{d in core's 512 head-dims} W_o[o, d] * y[d, q].
Host sums the 4 partials per batch (the all-reduce step of the hint,
performed at unshard time) and transposes to [S, H].

All matmuls run in bf16 with fp32 PSUM accumulation.

Structure (v2):
- Phase 1 streams xT ONCE (16 k-chunks x 4 S-quarters) and computes
  kv_d, q_d and raw rope-k together, tile-major per quarter so PSUM
  banks accumulate over the full contraction without re-streaming x.
- Softmax row-sums accumulate on the Pool engine (pair-sum + running
  fp32 add) with a single ones-matmul per (q-block, head) instead of
  one rowsum matmul per score chunk (saves ~30us of PE time).
- o_proj (lagged by one q-block) and the tail of the v up-projection
  are interleaved into the SDPA group loop so the PE has independent
  work while the scalar engine computes exp().
"""

import numpy as np
import ml_dtypes

import concourse.bass as bass
import concourse.mybir as mybir
import concourse.tile as tile
from concourse import bacc
from concourse._compat import get_trn_type
from concourse.bass_utils import run_bass_kernel_spmd

H = 2048
NH = 16
HD = 128           # head dim
RD = 64            # rotary dim
RH = 32            # rotary half
LAT = 256
B = 2
S = 2048
BASE = 10000.0
N_CORES = 8
HEADS_PER_CORE = 4
P = 128
NQB = S // 512     # 4 query blocks of 512
NKI = S // 128     # 16 key chunks of 128
SCALE = 1.0 / np.sqrt(float(HD))
EXP_BIAS = -4.0

BF16 = mybir.dt.bfloat16
F32 = mybir.dt.float32
_bf = ml_dtypes.bfloat16


def _mm(nc, out, lhsT, rhs, start, stop):
    nc.tensor.matmul(out, lhsT, rhs, start=start, stop=stop)


def build_program(nrep=1, bench_io=False):
    nc = bacc.Bacc(
        get_trn_type() or "TRN2",
        target_bir_lowering=False,
        debug=False,
        num_devices=N_CORES,
    )

    if bench_io:
        dummy = nc.declare_dram_parameter("bdummy", [1, 128], F32, isOutput=False)
        xT = nc.dram_tensor("xT", [H, S], BF16)
        w_kvd = nc.dram_tensor("w_kvd", [P, 16, LAT], BF16)
        w_qd = nc.dram_tensor("w_qd", [P, 16, LAT], BF16)
        w_rk = nc.dram_tensor("w_rk", [P, 16, 256], BF16)
        w_qc = nc.dram_tensor("w_qc", [P, 2, 512], BF16)
        w_kc = nc.dram_tensor("w_kc", [P, 2, 256], BF16)
        w_v = nc.dram_tensor("w_v", [P, 2, 512], BF16)
        w_o = nc.dram_tensor("w_o", [P, 4, H], BF16)
        cosA = nc.dram_tensor("cosA", [P, S], BF16)
        sinB = nc.dram_tensor("sinB", [P, S], BF16)
        masks = nc.dram_tensor("masks", [P, 4, 512], BF16)
        out = nc.dram_tensor("outs", [H, S], BF16)
        outp = nc.declare_dram_parameter("out", [1, 128], BF16, isOutput=True)
    else:
        xT = nc.declare_dram_parameter("xT", [H, S], BF16, isOutput=False)
        w_kvd = nc.declare_dram_parameter("w_kvd", [P, 16, LAT], BF16, isOutput=False)
        w_qd = nc.declare_dram_parameter("w_qd", [P, 16, LAT], BF16, isOutput=False)
        w_rk = nc.declare_dram_parameter("w_rk", [P, 16, 256], BF16, isOutput=False)
        w_qc = nc.declare_dram_parameter("w_qc", [P, 2, 512], BF16, isOutput=False)
        w_kc = nc.declare_dram_parameter("w_kc", [P, 2, 256], BF16, isOutput=False)
        w_v = nc.declare_dram_parameter("w_v", [P, 2, 512], BF16, isOutput=False)
        w_o = nc.declare_dram_parameter("w_o", [P, 4, H], BF16, isOutput=False)
        cosA = nc.declare_dram_parameter("cosA", [P, S], BF16, isOutput=False)
        sinB = nc.declare_dram_parameter("sinB", [P, S], BF16, isOutput=False)
        masks = nc.declare_dram_parameter("masks", [P, 4, 512], BF16, isOutput=False)
        out = nc.declare_dram_parameter("out", [H, S], BF16, isOutput=True)

    Exp = mybir.ActivationFunctionType.Exp

    scratch = (nc.dram_tensor("scratch", [H, S], BF16) if nrep > 1 else None)

    with tile.TileContext(nc) as tc:
      for rep in range(nrep):
        out_r = out if rep == nrep - 1 else scratch
        with (
            tc.tile_pool(name=f"wpool{rep}", bufs=1) as wpool,
            tc.tile_pool(name=f"main1_{rep}", bufs=1) as main1,
            tc.tile_pool(name=f"main2_{rep}", bufs=1) as main2,
            tc.tile_pool(name=f"ph1_{rep}", bufs=1) as ph1,
            tc.tile_pool(name=f"rot{rep}", bufs=3) as rot,
        ):
            # -------- persistent small tensors (DMAs deferred) --------
            cos_sb = wpool.tile([P, S], BF16, tag="cos", name="cos")
            sin_sb = wpool.tile([P, S], BF16, tag="sin", name="sin")
            mask_sb = wpool.tile([P, 4, 512], BF16, tag="mask", name="mask")
            ones_sb = wpool.tile([P, P], BF16, tag="ones", name="ones")
            nc.gpsimd.memset(ones_sb[:], 1.0)
            ebias_sb = wpool.tile([P, 1], F32, tag="ebias", name="ebias")
            nc.gpsimd.memset(ebias_sb[:], EXP_BIAS)
            wo_sb = wpool.tile([P, 4, H], BF16, tag="wo", name="wo")
            wqc_sb = wpool.tile([P, 2, 512], BF16, tag="wqc", name="wqc")
            wkc_sb = wpool.tile([P, 2, 256], BF16, tag="wkc", name="wkc")
            wv_sb = wpool.tile([P, 2, 512], BF16, tag="wv", name="wv")

            # -------- phase-1 outputs (latents), bf16 --------
            kvd_sb = [main1.tile([P, S], BF16, tag=f"kvd{m}", name=f"kvd{m}")
                      for m in range(2)]
            qd_sb = [main1.tile([P, S], BF16, tag=f"qd{m}", name=f"qd{m}")
                     for m in range(2)]

            # phase-1 weights, loaded in need-order pieces interleaved with
            # the first x chunks: chunk-major matmuls need chunk k of all
            # three weight tensors, so stream them k-range by k-range.
            wkvd_sb = ph1.tile([P, 16, LAT], BF16, tag="wkvd", name="wkvd")
            wqd_sb = ph1.tile([P, 16, LAT], BF16, tag="wqd", name="wqd")
            wrk_sb = ph1.tile([P, 16, 256], BF16, tag="wrk", name="wrk")
            # latent weights stream on the scalar queue in parallel with
            # the sync queue's wrk + x chunks for faster early ramp
            for ks in (slice(0, 1), slice(1, 2), slice(2, 4)):
                nc.sync.dma_start(wrk_sb[:, ks, :], w_rk[:, ks, :])
                nc.scalar.dma_start(wkvd_sb[:, ks, :], w_kvd[:, ks, :])
                nc.scalar.dma_start(wqd_sb[:, ks, :], w_qd[:, ks, :])

            # -------- per-head q/k tiles (dims on partitions), v, y --------
            # even head h: rows [0:64] content, [64:128] rope
            # odd  head h: rows [0:64] rope,    [64:128] content
            qT = [main2.tile([P, S], BF16, tag=f"qT{h}", name=f"qT{h}")
                  for h in range(4)]
            kT = [main2.tile([P, S], BF16, tag=f"kT{h}", name=f"kT{h}")
                  for h in range(4)]
            v_sb = [main2.tile([P, NKI, 256], BF16, tag=f"v{p}", name=f"v{p}")
                    for p in range(2)]
            y_sb = [main2.tile([P, S], BF16, tag=f"y{h}", name=f"y{h}")
                    for h in range(4)]

            # -------- phase 1: single pass over x, S in 4 quarters --------
            # Per quarter: 6 PSUM banks accumulate (kr0, kr1, kvd0, kvd1,
            # qd0, qd1) over the 16 k-chunks, chunk-major so the PE starts
            # as soon as the first weight pieces + x chunk land.
            with tc.tile_pool(name=f"ps1_{rep}", bufs=1, space="PSUM") as ps1:
                for n in range(4):
                    qs = slice(n * 512, (n + 1) * 512)
                    xq = ph1.tile([P, NKI, 512], BF16, tag="xq", name="xq",
                                  bufs=2)
                    for k in range(NKI):
                        nc.sync.dma_start(xq[:, k, :],
                                          xT[k * 128:(k + 1) * 128, qs])
                        if n == 0 and k in (1, 3):
                            # tail of the phase-1 weights in need-order,
                            # spread across three queues so the x stream
                            # keeps its share of the DMA engines
                            ks = slice(4, 8) if k == 1 else slice(8, 16)
                            nc.gpsimd.dma_start(wrk_sb[:, ks, :],
                                                w_rk[:, ks, :])
                            nc.scalar.dma_start(wkvd_sb[:, ks, :],
                                                w_kvd[:, ks, :])
                            nc.scalar.dma_start(wqd_sb[:, ks, :],
                                                w_qd[:, ks, :])
                    if n == 0:
                        # cos/sin feed quarter-0's rope: load them on the
                        # gpsimd queue so the x stream keeps the sync queue
                        nc.gpsimd.dma_start(cos_sb[:], cosA[:, :])
                        nc.gpsimd.dma_start(sin_sb[:], sinB[:, :])
                    elif n == 1:
                        nc.sync.dma_start(mask_sb[:], masks[:, :, :])
                        nc.sync.dma_start(wqc_sb[:], w_qc[:, :, :])
                        nc.sync.dma_start(wkc_sb[:], w_kc[:, :, :])
                        nc.sync.dma_start(wv_sb[:], w_v[:, :, :])
                    elif n == 2:
                        nc.sync.dma_start(wo_sb[:], w_o[:, :, :])

                    krb = [ps1.tile([P, 512], F32, tag=f"kr{p}",
                                    name=f"kr{p}", bufs=2) for p in range(2)]
                    lat = [ps1.tile([P, 512], F32, tag=f"lat{i}",
                                    name=f"lat{i}") for i in range(4)]
                    plan = [
                        (wrk_sb, 0, krb[0]),
                        (wrk_sb, 1, krb[1]),
                        (wkvd_sb, 0, lat[0]),
                        (wkvd_sb, 1, lat[1]),
                        (wqd_sb, 0, lat[2]),
                        (wqd_sb, 1, lat[3]),
                    ]
                    for k in range(NKI):
                        for wsb, mi, bank in plan:
                            _mm(nc, bank[:],
                                wsb[:, k, mi * 128:(mi + 1) * 128],
                                xq[:, k, :], k == 0, k == NKI - 1)

                    # latent copies first (their banks gate the next
                    # quarter; kraw banks are double-buffered); split
                    # across Scalar and Vector to halve the drain
                    targets = [kvd_sb[0], kvd_sb[1], qd_sb[0], qd_sb[1]]
                    for i, tgt in enumerate(targets):
                        if i % 2 == 0:
                            nc.scalar.copy(tgt[:, qs], lat[i][:])
                        else:
                            nc.vector.tensor_copy(tgt[:, qs], lat[i][:])

                    # rope on raw k for this quarter:
                    # krb[p] rows [0:64]=h1 rope, [64:128]=h0 rope
                    for p in range(2):
                        h0, h1 = 2 * p, 2 * p + 1
                        kraw = rot.tile([P, 512], BF16, tag="krawc",
                                        name="krawc", bufs=3)
                        if n == 3:
                            # last quarter: keep Scalar free for the
                            # phase-2 copies that gate the next matmuls
                            nc.vector.tensor_copy(kraw[:], krb[p][:])
                        else:
                            nc.scalar.copy(kraw[:], krb[p][:])
                        swp = rot.tile([P, 512], BF16, tag="swp", name="swp",
                                       bufs=3)
                        nc.gpsimd.dma_start(swp[0:32, :], kraw[32:64, :])
                        nc.gpsimd.dma_start(swp[32:64, :], kraw[0:32, :])
                        nc.gpsimd.dma_start(swp[64:96, :], kraw[96:128, :])
                        nc.gpsimd.dma_start(swp[96:128, :], kraw[64:96, :])
                        nc.gpsimd.tensor_mul(swp[:], swp[:], sin_sb[:, qs])
                        nc.vector.tensor_mul(
                            kT[h1][0:64, qs], kraw[0:64, :], cos_sb[0:64, qs]
                        )
                        nc.vector.tensor_add(
                            kT[h1][0:64, qs], kT[h1][0:64, qs], swp[0:64, :]
                        )
                        nc.vector.tensor_mul(
                            kT[h0][64:128, qs], kraw[64:128, :],
                            cos_sb[64:128, qs]
                        )
                        nc.vector.tensor_add(
                            kT[h0][64:128, qs], kT[h0][64:128, qs],
                            swp[64:128, :]
                        )

            # -------- phase 2: q up-proj + rope (head-major, so the DVE
            # rope chain for head h overlaps later heads' matmuls and the
            # first SDPA blocks), k content (copies on Pool), v head --------
            with tc.tile_pool(name=f"ps2_{rep}", bufs=2, space="PSUM") as ps2:
                def qcomb_head(h):
                    for n2 in range(4):
                        ns = slice(n2 * 512, (n2 + 1) * 512)
                        pt = ps2.tile([P, 512], F32, tag="qcomb",
                                      name="qcomb", bufs=3)
                        for kc in range(2):
                            _mm(nc, pt[:],
                                wqc_sb[:, kc, h * 128:(h + 1) * 128],
                                qd_sb[kc][:, ns], kc == 0, kc == 1)
                        nc.scalar.copy(qT[h][:, ns], pt[:])
                    r = 64 if h % 2 == 0 else 0
                    swp = rot.tile([P, S], BF16, tag="swpq", name="swpq",
                                   bufs=2)
                    nc.gpsimd.dma_start(swp[r:r + 32, :],
                                        qT[h][r + 32:r + 64, :])
                    nc.gpsimd.dma_start(swp[r + 32:r + 64, :],
                                        qT[h][r:r + 32, :])
                    nc.gpsimd.tensor_mul(
                        swp[r:r + 64, :], swp[r:r + 64, :], sin_sb[r:r + 64, :]
                    )
                    nc.vector.tensor_mul(
                        qT[h][r:r + 64, :], qT[h][r:r + 64, :],
                        cos_sb[r:r + 64, :]
                    )
                    nc.vector.tensor_add(
                        qT[h][r:r + 64, :], qT[h][r:r + 64, :], swp[r:r + 64, :]
                    )

                def kc_pair(p2):
                    # k content: psum rows [0:64]=h0, [64:128]=h1
                    h0, h1 = 2 * p2, 2 * p2 + 1
                    for n2 in range(4):
                        ns = slice(n2 * 512, (n2 + 1) * 512)
                        pt = ps2.tile([P, 512], F32, tag="kcont",
                                      name="kcont")
                        for kc in range(2):
                            _mm(nc, pt[:],
                                wkc_sb[:, kc, p2 * 128:(p2 + 1) * 128],
                                kvd_sb[kc][:, ns], kc == 0, kc == 1)
                        nc.scalar.copy(kT[h0][0:64, ns], pt[0:64, :])
                        nc.vector.tensor_copy(kT[h1][64:128, ns],
                                              pt[64:128, :])

                def v_first(p2):
                    # v for the first 8 key chunks (the rest is interleaved
                    # into the SDPA loop as PE filler work); phase 2 has
                    # spare PE time, so front-load the matmuls here
                    for s16 in range(8):
                        pt = ps2.tile([P, 256], F32, tag="vps", name="vps")
                        for kc in range(2):
                            _mm(nc, pt[:],
                                kvd_sb[kc][:, s16 * 128:(s16 + 1) * 128],
                                wv_sb[:, kc, p2 * 256:(p2 + 1) * 256],
                                kc == 0, kc == 1)
                        if s16 % 2 == 0:
                            nc.scalar.copy(v_sb[p2][:, s16, :], pt[:])
                        else:
                            nc.vector.tensor_copy(v_sb[p2][:, s16, :], pt[:])

                kc_pair(0)
                qcomb_head(0)
                v_first(0)
                qcomb_head(1)
                kc_pair(1)
                qcomb_head(2)
                v_first(1)
                qcomb_head(3)

            # -------- SDPA + interleaved o_proj / v tail --------
            with tc.tile_pool(name=f"ps3_{rep}", bufs=1, space="PSUM") as ps3:
                pend_v = [(s16, p) for s16 in range(8, NKI) for p in range(2)]
                pend_o = []

                def emit_v(cast_scalar=False):
                    s16, p = pend_v.pop(0)
                    pt = ps3.tile([P, 512], F32, tag="aux", name="vaux",
                                  bufs=2)
                    for kc in range(2):
                        _mm(nc, pt[:, 0:256],
                            kvd_sb[kc][:, s16 * 128:(s16 + 1) * 128],
                            wv_sb[:, kc, p * 256:(p + 1) * 256],
                            kc == 0, kc == 1)
                    if cast_scalar:
                        nc.scalar.copy(v_sb[p][:, s16, :], pt[:, 0:256])
                    else:
                        nc.vector.tensor_copy(v_sb[p][:, s16, :],
                                              pt[:, 0:256])

                def emit_o(cast_scalar=False):
                    qb_o, oc = pend_o.pop(0)
                    qs_o = slice(qb_o * 512, (qb_o + 1) * 512)
                    opt_ = ps3.tile([P, 512], F32, tag="aux", name="opj",
                                    bufs=2)
                    for hk in range(4):
                        _mm(nc, opt_[:],
                            wo_sb[:, hk, oc * 128:(oc + 1) * 128],
                            y_sb[hk][:, qs_o], hk == 0, hk == 3)
                    osb = rot.tile([P, 512], BF16, tag="osb", name="osb",
                                   bufs=4)
                    if cast_scalar:
                        nc.scalar.copy(osb[:], opt_[:])
                        nc.scalar.dma_start(
                            out_r[oc * 128:(oc + 1) * 128, qs_o], osb[:])
                    else:
                        nc.vector.tensor_copy(osb[:], opt_[:])
                        nc.sync.dma_start(
                            out_r[oc * 128:(oc + 1) * 128, qs_o], osb[:])

                for qb in range(NQB):
                    qs = slice(qb * 512, (qb + 1) * 512)
                    for h in range(4):
                        p = h // 2
                        hv = (h % 2) * 128
                        ngr = 2 * (qb + 1)    # groups of 2 k-chunks
                        yps = ps3.tile([P, 512], F32, tag="ypv", name="ypv",
                                       bufs=1)
                        rsps = ps3.tile([P, 512], F32, tag="rs", name="rs",
                                        bufs=1)
                        for g in range(ngr):
                            scps = ps3.tile([P, 2, 512], F32, tag="sc",
                                            name="sc", bufs=2)
                            for j in range(2):
                                ki = 2 * g + j
                                _mm(nc, scps[:, j, :],
                                    kT[h][:, ki * 128:(ki + 1) * 128],
                                    qT[h][:, qs], True, True)
                            prb = rot.tile([P, 2, 512], BF16, tag="prb",
                                           name="prb", bufs=3)
                            nc.scalar.activation(
                                prb[:], scps[:], Exp, bias=ebias_sb[:],
                                scale=SCALE
                            )
                            for j in range(2):
                                ki = 2 * g + j
                                if g // 2 == qb:
                                    # diagonal chunk: only a 128-wide
                                    # triangle needs masking, and all
                                    # columns left of it are excluded
                                    # from the AV/rowsum matmuls instead
                                    # of being zeroed
                                    dd = ki - 4 * qb
                                    ds = slice(dd * 128, (dd + 1) * 128)
                                    nc.vector.tensor_mul(
                                        prb[:, j, ds], prb[:, j, ds],
                                        mask_sb[:, dd, ds]
                                    )
                                    cs = slice(dd * 128, 512)
                                else:
                                    cs = slice(0, 512)
                                first = ki == 0
                                last = ki == 2 * ngr - 1
                                _mm(nc, yps[:, cs],
                                    v_sb[p][:, ki, hv:hv + 128],
                                    prb[:, j, cs], first, last)
                                _mm(nc, rsps[:, cs], ones_sb[:],
                                    prb[:, j, cs], first, last)
                            # independent PE filler while exp runs
                            if pend_v and qb <= 1:
                                emit_v(cast_scalar=len(pend_v) % 2 == 0)
                                if pend_v:
                                    emit_v(cast_scalar=len(pend_v) % 2 == 0)
                            elif pend_o and not (h == 0 and g < 2) and \
                                    (qb < 3 or (h * ngr + g) % 2 == 0):
                                emit_o()
                        rcp = rot.tile([P, 512], F32, tag="rcp", name="rcp",
                                       bufs=2)
                        nc.vector.reciprocal_approx_fast(rcp[:], rsps[:])
                        nc.vector.tensor_mul(y_sb[h][:, qs], yps[:], rcp[:])
                    pend_o.extend((qb, oc) for oc in range(16))
                flip = False
                while pend_o:
                    emit_o(cast_scalar=flip)
                    flip = not flip

      if bench_io:
          with tc.tile_pool(name="bo", bufs=1) as bo:
              bt = bo.tile([1, 128], BF16, tag="bt", name="bt")
              nc.sync.dma_start(bt[:], out[0:1, 0:128])
              nc.sync.dma_start(outp[:, :], bt[:])

    nc.compile()
    return nc


_NC = None


def _get_nc():
    global _NC
    if _NC is None:
        _NC = build_program()
    return _NC


def _rope_tables():
    """cosA/sinB [128, S]: 32-row frequency pattern tiled 4x.
    sinB sign: rows [0:32] of each 64-block -> -sin, rows [32:64] -> +sin."""
    inv_freq = 1.0 / (BASE ** (np.arange(0, RD, 2, dtype=np.float32) / RD))  # [32]
    pos = np.arange(S, dtype=np.float32)
    ang = inv_freq[:, None] * pos[None, :]              # [32, S]
    cos1, sin1 = np.cos(ang), np.sin(ang)
    cosA = np.tile(cos1, (4, 1))                        # [128, S]
    sinB = np.concatenate([-sin1, sin1, -sin1, sin1], axis=0)
    return cosA.astype(_bf), sinB.astype(_bf)


def _mask_tiles():
    """masks[d][k, q] = 1.0 if q >= d*128 + k else 0 (bf16, [4,128,512])."""
    k = np.arange(P)[:, None]
    q = np.arange(512)[None, :]
    m = np.stack([(q >= d * 128 + k) for d in range(4)]).astype(np.float32)
    return np.ascontiguousarray(m.transpose(1, 0, 2)).astype(_bf)


def _prep_core_inputs(c, x, W_kv_d, W_q_d, W_k_u, W_q_u, W_v_u, W_rope_k, W_rope_q,
                      W_o, cosA, sinB, masks):
    b = c // 4
    hg = c % 4
    heads = [4 * hg + j for j in range(HEADS_PER_CORE)]

    def tile_pmaj(w):
        # [ko*128, m] -> [128, ko, m] partition-major for contiguous DMA
        ko = w.shape[0] // P
        return np.ascontiguousarray(
            w.reshape(ko, P, w.shape[1]).transpose(1, 0, 2))

    xT = np.ascontiguousarray(x[b].T).astype(_bf)                  # [H, S]
    w_kvd = tile_pmaj(np.ascontiguousarray(W_kv_d.T).astype(_bf))
    w_qd = tile_pmaj(np.ascontiguousarray(W_q_d.T).astype(_bf))

    # w_rk: per pair, rows [h1 rope dims | h0 rope dims], then transpose
    blocks = []
    for p in range(2):
        g0, g1 = heads[2 * p], heads[2 * p + 1]
        blocks.append(W_rope_k[g1 * RD:(g1 + 1) * RD, :])
        blocks.append(W_rope_k[g0 * RD:(g0 + 1) * RD, :])
    w_rk = tile_pmaj(np.ascontiguousarray(np.concatenate(blocks, axis=0).T).astype(_bf))

    # w_qc: per local head 128 cols: even -> [content|rope], odd -> [rope|content]
    cols = []
    for j, g in enumerate(heads):
        c_blk = W_q_u[g * RD:(g + 1) * RD, :].T       # [LAT, 64]
        r_blk = W_rope_q[g * RD:(g + 1) * RD, :].T    # [LAT, 64]
        cols.extend([c_blk, r_blk] if j % 2 == 0 else [r_blk, c_blk])
    w_qc = tile_pmaj(np.ascontiguousarray(np.concatenate(cols, axis=1)).astype(_bf))

    # w_kc: per pair 128 cols: [h0 content | h1 content]
    cols = []
    for p in range(2):
        g0, g1 = heads[2 * p], heads[2 * p + 1]
        cols.append(W_k_u[g0 * RD:(g0 + 1) * RD, :].T)
        cols.append(W_k_u[g1 * RD:(g1 + 1) * RD, :].T)
    w_kc = tile_pmaj(np.ascontiguousarray(np.concatenate(cols, axis=1)).astype(_bf))

    # w_v: per pair 256 cols: [h0 v dims | h1 v dims]
    cols = []
    for p in range(2):
        g0, g1 = heads[2 * p], heads[2 * p + 1]
        cols.append(W_v_u[g0 * HD:(g0 + 1) * HD, :].T)
        cols.append(W_v_u[g1 * HD:(g1 + 1) * HD, :].T)
    w_v = tile_pmaj(np.ascontiguousarray(np.concatenate(cols, axis=1)).astype(_bf))

    d0 = heads[0] * HD
    w_o = tile_pmaj(np.ascontiguousarray(W_o[:, d0:d0 + 512].T).astype(_bf))

    return {
        "xT": xT, "w_kvd": w_kvd, "w_qd": w_qd, "w_rk": w_rk, "w_qc": w_qc,
        "w_kc": w_kc, "w_v": w_v, "w_o": w_o, "cosA": cosA, "sinB": sinB,
        "masks": masks,
    }


def make_in_maps(inputs):
    x = np.asarray(inputs["hidden_states"], dtype=np.float32)
    ws = {k: np.asarray(inputs[k], dtype=np.float32)
          for k in ("W_kv_d", "W_q_d", "W_k_u", "W_q_u", "W_v_u", "W_rope_k",
                    "W_rope_q", "W_o")}
    cosA, sinB = _rope_tables()
    masks = _mask_tiles()
    return [
        _prep_core_inputs(c, x, ws["W_kv_d"], ws["W_q_d"], ws["W_k_u"],
                          ws["W_q_u"], ws["W_v_u"], ws["W_rope_k"],
                          ws["W_rope_q"], ws["W_o"], cosA, sinB, masks)
        for c in range(N_CORES)
    ]


def assemble(results):
    """results: list of 8 dicts with 'out' [H, S] f32 partials (transposed)."""
    full = np.empty((B, S, H), dtype=np.float32)
    for b in range(B):
        acc = results[4 * b]["out"].astype(np.float32)
        for g in range(1, 4):
            acc = acc + results[4 * b + g]["out"]
        full[b] = acc.T
    return full


def kernel(**inputs):
    nc = _get_nc()
    in_maps = make_in_maps(inputs)
    res = run_bass_kernel_spmd(nc, in_maps, core_ids=list(range(N_CORES)))
    return assemble(res.results)


# revision 48
# speedup vs baseline: 1.0208x; 1.0208x over previous
"""MLA (multi-head latent attention) Trainium2 kernel.

Sharding: 8 cores = 2 (batch) x 4 (head groups of 4 heads).
Each core computes, for its batch b and heads [4g, 4g+4):
  latents kv_d/q_d (replicated within the batch group), per-head
  up-projections + RoPE, causal SDPA, and a partial o_proj
  out_core[o, q] = sum_{d in core's 512 head-dims} W_o[o, d] * y[d, q].
Host sums the 4 partials per batch (the all-reduce step of the hint,
performed at unshard time) and transposes to [S, H].

All matmuls run in bf16 with fp32 PSUM accumulation.

Structure:
- Phase 1 streams xT ONCE (16 k-chunks x 4 S-quarters, chunk-major)
  and computes kv_d, q_d and raw rope-k together; weight pieces are
  loaded in need-order across two DMA queues so the PE starts early.
- PSUM->SBUF copies are spread across Scalar and Vector; rope swap
  transposes issue on the idle gpsimd DMA queue.
- SDPA: for diagonal (causal-boundary) key chunks only a 128-wide
  triangle is masked and all columns left of it are excluded from the
  AV/rowsum matmuls (narrower free dim) instead of being zeroed.
- o_proj (lagged by one q-block, paced every other group in the last
  block) and the tail of the v up-projection are interleaved into the
  SDPA group loop so the PE has independent work while the scalar
  engine computes exp().
"""

import numpy as np
import ml_dtypes

import concourse.bass as bass
import concourse.mybir as mybir
import concourse.tile as tile
from concourse import bacc
from concourse._compat import get_trn_type
from concourse.bass_utils import run_bass_kernel_spmd

H = 2048
NH = 16
HD = 128           # head dim
RD = 64            # rotary dim
RH = 32            # rotary half
LAT = 256
B = 2
S = 2048
BASE = 10000.0
N_CORES = 8
HEADS_PER_CORE = 4
P = 128
NQB = S // 512     # 4 query blocks of 512
NKI = S // 128     # 16 key chunks of 128
SCALE = 1.0 / np.sqrt(float(HD))
EXP_BIAS = -4.0

BF16 = mybir.dt.bfloat16
F32 = mybir.dt.float32
_bf = ml_dtypes.bfloat16


def _mm(nc, out, lhsT, rhs, start, stop):
    nc.tensor.matmul(out, lhsT, rhs, start=start, stop=stop)


def build_program(nrep=1, bench_io=False):
    nc = bacc.Bacc(
        get_trn_type() or "TRN2",
        target_bir_lowering=False,
        debug=False,
        num_devices=N_CORES,
    )

    if bench_io:
        dummy = nc.declare_dram_parameter("bdummy", [1, 128], F32, isOutput=False)
        xT = nc.dram_tensor("xT", [H, S], BF16)
        w_kvd = nc.dram_tensor("w_kvd", [P, 16, LAT], BF16)
        w_qd = nc.dram_tensor("w_qd", [P, 16, LAT], BF16)
        w_rk = nc.dram_tensor("w_rk", [P, 16, 256], BF16)
        w_qc = nc.dram_tensor("w_qc", [P, 2, 512], BF16)
        w_kc = nc.dram_tensor("w_kc", [P, 2, 256], BF16)
        w_v = nc.dram_tensor("w_v", [P, 2, 512], BF16)
        w_o = nc.dram_tensor("w_o", [P, 4, H], BF16)
        cosA = nc.dram_tensor("cosA", [P, S], BF16)
        sinB = nc.dram_tensor("sinB", [P, S], BF16)
        masks = nc.dram_tensor("masks", [P, 4, 512], BF16)
        out = nc.dram_tensor("outs", [H, S], BF16)
        outp = nc.declare_dram_parameter("out", [1, 128], BF16, isOutput=True)
    else:
        xT = nc.declare_dram_parameter("xT", [H, S], BF16, isOutput=False)
        w_kvd = nc.declare_dram_parameter("w_kvd", [P, 16, LAT], BF16, isOutput=False)
        w_qd = nc.declare_dram_parameter("w_qd", [P, 16, LAT], BF16, isOutput=False)
        w_rk = nc.declare_dram_parameter("w_rk", [P, 16, 256], BF16, isOutput=False)
        w_qc = nc.declare_dram_parameter("w_qc", [P, 2, 512], BF16, isOutput=False)
        w_kc = nc.declare_dram_parameter("w_kc", [P, 2, 256], BF16, isOutput=False)
        w_v = nc.declare_dram_parameter("w_v", [P, 2, 512], BF16, isOutput=False)
        w_o = nc.declare_dram_parameter("w_o", [P, 4, H], BF16, isOutput=False)
        cosA = nc.declare_dram_parameter("cosA", [P, S], BF16, isOutput=False)
        sinB = nc.declare_dram_parameter("sinB", [P, S], BF16, isOutput=False)
        masks = nc.declare_dram_parameter("masks", [P, 4, 512], BF16, isOutput=False)
        out = nc.declare_dram_parameter("out", [H, S], BF16, isOutput=True)

    Exp = mybir.ActivationFunctionType.Exp

    scratch = (nc.dram_tensor("scratch", [H, S], BF16) if nrep > 1 else None)

    with tile.TileContext(nc) as tc:
      for rep in range(nrep):
        out_r = out if rep == nrep - 1 else scratch
        with (
            tc.tile_pool(name=f"wpool{rep}", bufs=1) as wpool,
            tc.tile_pool(name=f"main1_{rep}", bufs=1) as main1,
            tc.tile_pool(name=f"main2_{rep}", bufs=1) as main2,
            tc.tile_pool(name=f"ph1_{rep}", bufs=1) as ph1,
            tc.tile_pool(name=f"rot{rep}", bufs=3) as rot,
        ):
            # -------- persistent small tensors (DMAs deferred) --------
            cos_sb = wpool.tile([P, S], BF16, tag="cos", name="cos")
            sin_sb = wpool.tile([P, S], BF16, tag="sin", name="sin")
            mask_sb = wpool.tile([P, 4, 512], BF16, tag="mask", name="mask")
            ones_sb = wpool.tile([P, P], BF16, tag="ones", name="ones")
            nc.gpsimd.memset(ones_sb[:], 1.0)
            ebias_sb = wpool.tile([P, 1], F32, tag="ebias", name="ebias")
            nc.gpsimd.memset(ebias_sb[:], EXP_BIAS)
            wo_sb = wpool.tile([P, 4, H], BF16, tag="wo", name="wo")
            wqc_sb = wpool.tile([P, 2, 512], BF16, tag="wqc", name="wqc")
            wkc_sb = wpool.tile([P, 2, 256], BF16, tag="wkc", name="wkc")
            wv_sb = wpool.tile([P, 2, 512], BF16, tag="wv", name="wv")

            # -------- phase-1 outputs (latents), bf16 --------
            kvd_sb = [main1.tile([P, S], BF16, tag=f"kvd{m}", name=f"kvd{m}")
                      for m in range(2)]
            qd_sb = [main1.tile([P, S], BF16, tag=f"qd{m}", name=f"qd{m}")
                     for m in range(2)]

            # phase-1 weights, loaded in need-order pieces interleaved with
            # the first x chunks: chunk-major matmuls need chunk k of all
            # three weight tensors, so stream them k-range by k-range.
            wkvd_sb = ph1.tile([P, 16, LAT], BF16, tag="wkvd", name="wkvd")
            wqd_sb = ph1.tile([P, 16, LAT], BF16, tag="wqd", name="wqd")
            wrk_sb = ph1.tile([P, 16, 256], BF16, tag="wrk", name="wrk")
            # latent weights stream on the scalar queue in parallel with
            # the sync queue's wrk + x chunks for faster early ramp
            for ks in (slice(0, 1), slice(1, 2), slice(2, 4)):
                nc.sync.dma_start(wrk_sb[:, ks, :], w_rk[:, ks, :])
                nc.scalar.dma_start(wkvd_sb[:, ks, :], w_kvd[:, ks, :])
                nc.scalar.dma_start(wqd_sb[:, ks, :], w_qd[:, ks, :])

            # -------- per-head q/k tiles (dims on partitions), v, y --------
            # even head h: rows [0:64] content, [64:128] rope
            # odd  head h: rows [0:64] rope,    [64:128] content
            qT = [main2.tile([P, S], BF16, tag=f"qT{h}", name=f"qT{h}")
                  for h in range(4)]
            kT = [main2.tile([P, S], BF16, tag=f"kT{h}", name=f"kT{h}")
                  for h in range(4)]
            v_sb = [main2.tile([P, NKI, 256], BF16, tag=f"v{p}", name=f"v{p}")
                    for p in range(2)]
            y_sb = [main2.tile([P, S], BF16, tag=f"y{h}", name=f"y{h}")
                    for h in range(4)]

            # -------- phase 1: single pass over x, S in 4 quarters --------
            # Per quarter: 6 PSUM banks accumulate (kr0, kr1, kvd0, kvd1,
            # qd0, qd1) over the 16 k-chunks, chunk-major so the PE starts
            # as soon as the first weight pieces + x chunk land.
            with tc.tile_pool(name=f"ps1_{rep}", bufs=1, space="PSUM") as ps1:
                for n in range(4):
                    qs = slice(n * 512, (n + 1) * 512)
                    xq = ph1.tile([P, NKI, 512], BF16, tag="xq", name="xq",
                                  bufs=2)
                    for k in range(NKI):
                        nc.sync.dma_start(xq[:, k, :],
                                          xT[k * 128:(k + 1) * 128, qs])
                        if n == 0 and k in (1, 3):
                            # tail of the phase-1 weights in need-order,
                            # spread across three queues so the x stream
                            # keeps its share of the DMA engines
                            ks = slice(4, 8) if k == 1 else slice(8, 16)
                            nc.sync.dma_start(wrk_sb[:, ks, :], w_rk[:, ks, :])
                            nc.scalar.dma_start(wkvd_sb[:, ks, :],
                                                w_kvd[:, ks, :])
                            nc.scalar.dma_start(wqd_sb[:, ks, :],
                                                w_qd[:, ks, :])
                    if n == 0:
                        nc.sync.dma_start(cos_sb[:], cosA[:, :])
                        nc.sync.dma_start(sin_sb[:], sinB[:, :])
                    elif n == 1:
                        nc.sync.dma_start(mask_sb[:], masks[:, :, :])
                        nc.sync.dma_start(wqc_sb[:], w_qc[:, :, :])
                        nc.sync.dma_start(wkc_sb[:], w_kc[:, :, :])
                        nc.sync.dma_start(wv_sb[:], w_v[:, :, :])
                    elif n == 2:
                        nc.sync.dma_start(wo_sb[:], w_o[:, :, :])

                    krb = [ps1.tile([P, 512], F32, tag=f"kr{p}",
                                    name=f"kr{p}", bufs=2) for p in range(2)]
                    lat = [ps1.tile([P, 512], F32, tag=f"lat{i}",
                                    name=f"lat{i}") for i in range(4)]
                    plan = [
                        (wrk_sb, 0, krb[0]),
                        (wrk_sb, 1, krb[1]),
                        (wkvd_sb, 0, lat[0]),
                        (wkvd_sb, 1, lat[1]),
                        (wqd_sb, 0, lat[2]),
                        (wqd_sb, 1, lat[3]),
                    ]
                    for k in range(NKI):
                        for wsb, mi, bank in plan:
                            _mm(nc, bank[:],
                                wsb[:, k, mi * 128:(mi + 1) * 128],
                                xq[:, k, :], k == 0, k == NKI - 1)

                    # latent copies first (their banks gate the next
                    # quarter; kraw banks are double-buffered); split
                    # across Scalar and Vector to halve the drain
                    targets = [kvd_sb[0], kvd_sb[1], qd_sb[0], qd_sb[1]]
                    for i, tgt in enumerate(targets):
                        if i < 2:
                            nc.scalar.copy(tgt[:, qs], lat[i][:])
                        else:
                            nc.vector.tensor_copy(tgt[:, qs], lat[i][:])

                    # rope on raw k for this quarter:
                    # krb[p] rows [0:64]=h1 rope, [64:128]=h0 rope
                    for p in range(2):
                        h0, h1 = 2 * p, 2 * p + 1
                        kraw = rot.tile([P, 512], BF16, tag="krawc",
                                        name="krawc", bufs=3)
                        if n == 3:
                            # last quarter: keep Scalar free for the
                            # phase-2 copies that gate the next matmuls
                            nc.vector.tensor_copy(kraw[:], krb[p][:])
                        else:
                            nc.scalar.copy(kraw[:], krb[p][:])
                        swp = rot.tile([P, 512], BF16, tag="swp", name="swp",
                                       bufs=3)
                        nc.gpsimd.dma_start(swp[0:32, :], kraw[32:64, :])
                        nc.gpsimd.dma_start(swp[32:64, :], kraw[0:32, :])
                        nc.gpsimd.dma_start(swp[64:96, :], kraw[96:128, :])
                        nc.gpsimd.dma_start(swp[96:128, :], kraw[64:96, :])
                        nc.vector.tensor_mul(swp[:], swp[:], sin_sb[:, qs])
                        nc.vector.tensor_mul(
                            kT[h1][0:64, qs], kraw[0:64, :], cos_sb[0:64, qs]
                        )
                        nc.vector.tensor_add(
                            kT[h1][0:64, qs], kT[h1][0:64, qs], swp[0:64, :]
                        )
                        nc.vector.tensor_mul(
                            kT[h0][64:128, qs], kraw[64:128, :],
                            cos_sb[64:128, qs]
                        )
                        nc.vector.tensor_add(
                            kT[h0][64:128, qs], kT[h0][64:128, qs],
                            swp[64:128, :]
                        )

            # -------- phase 2: q up-proj + rope (head-major, so the DVE
            # rope chain for head h overlaps later heads' matmuls and the
            # first SDPA blocks), k content (copies on Pool), v head --------
            with tc.tile_pool(name=f"ps2_{rep}", bufs=2, space="PSUM") as ps2:
                def qcomb_head(h):
                    for n2 in range(4):
                        ns = slice(n2 * 512, (n2 + 1) * 512)
                        pt = ps2.tile([P, 512], F32, tag="qcomb",
                                      name="qcomb", bufs=3)
                        for kc in range(2):
                            _mm(nc, pt[:],
                                wqc_sb[:, kc, h * 128:(h + 1) * 128],
                                qd_sb[kc][:, ns], kc == 0, kc == 1)
                        nc.scalar.copy(qT[h][:, ns], pt[:])
                    r = 64 if h % 2 == 0 else 0
                    swp = rot.tile([P, S], BF16, tag="swpq", name="swpq",
                                   bufs=2)
                    nc.sync.dma_start(swp[r:r + 32, :],
                                      qT[h][r + 32:r + 64, :])
                    nc.sync.dma_start(swp[r + 32:r + 64, :],
                                      qT[h][r:r + 32, :])
                    nc.vector.tensor_mul(
                        qT[h][r:r + 64, :], qT[h][r:r + 64, :],
                        cos_sb[r:r + 64, :]
                    )
                    nc.vector.tensor_mul(
                        swp[r:r + 64, :], swp[r:r + 64, :], sin_sb[r:r + 64, :]
                    )
                    nc.vector.tensor_add(
                        qT[h][r:r + 64, :], qT[h][r:r + 64, :], swp[r:r + 64, :]
                    )

                def kc_pair(p2):
                    # k content: psum rows [0:64]=h0, [64:128]=h1
                    h0, h1 = 2 * p2, 2 * p2 + 1
                    for n2 in range(4):
                        ns = slice(n2 * 512, (n2 + 1) * 512)
                        pt = ps2.tile([P, 512], F32, tag="kcont",
                                      name="kcont")
                        for kc in range(2):
                            _mm(nc, pt[:],
                                wkc_sb[:, kc, p2 * 128:(p2 + 1) * 128],
                                kvd_sb[kc][:, ns], kc == 0, kc == 1)
                        nc.scalar.copy(kT[h0][0:64, ns], pt[0:64, :])
                        nc.vector.tensor_copy(kT[h1][64:128, ns],
                                              pt[64:128, :])

                def v_first(p2):
                    # v for the first 8 key chunks (the rest is interleaved
                    # into the SDPA loop as PE filler work); phase 2 has
                    # spare PE time, so front-load the matmuls here
                    for s16 in range(8):
                        pt = ps2.tile([P, 256], F32, tag="vps", name="vps")
                        for kc in range(2):
                            _mm(nc, pt[:],
                                kvd_sb[kc][:, s16 * 128:(s16 + 1) * 128],
                                wv_sb[:, kc, p2 * 256:(p2 + 1) * 256],
                                kc == 0, kc == 1)
                        if s16 % 2 == 0:
                            nc.scalar.copy(v_sb[p2][:, s16, :], pt[:])
                        else:
                            nc.vector.tensor_copy(v_sb[p2][:, s16, :], pt[:])

                qcomb_head(0)
                kc_pair(0)
                qcomb_head(1)
                v_first(0)
                kc_pair(1)
                qcomb_head(2)
                v_first(1)
                qcomb_head(3)

            # -------- SDPA + interleaved o_proj / v tail --------
            with tc.tile_pool(name=f"ps3_{rep}", bufs=1, space="PSUM") as ps3:
                pend_v = [(s16, p) for s16 in range(8, NKI) for p in range(2)]
                pend_o = []

                def emit_v(cast_scalar=False):
                    s16, p = pend_v.pop(0)
                    pt = ps3.tile([P, 512], F32, tag="aux", name="vaux",
                                  bufs=2)
                    for kc in range(2):
                        _mm(nc, pt[:, 0:256],
                            kvd_sb[kc][:, s16 * 128:(s16 + 1) * 128],
                            wv_sb[:, kc, p * 256:(p + 1) * 256],
                            kc == 0, kc == 1)
                    if cast_scalar:
                        nc.scalar.copy(v_sb[p][:, s16, :], pt[:, 0:256])
                    else:
                        nc.vector.tensor_copy(v_sb[p][:, s16, :],
                                              pt[:, 0:256])

                def emit_o(cast_scalar=False):
                    qb_o, oc = pend_o.pop(0)
                    qs_o = slice(qb_o * 512, (qb_o + 1) * 512)
                    opt_ = ps3.tile([P, 512], F32, tag="aux", name="opj",
                                    bufs=2)
                    for hk in range(4):
                        _mm(nc, opt_[:],
                            wo_sb[:, hk, oc * 128:(oc + 1) * 128],
                            y_sb[hk][:, qs_o], hk == 0, hk == 3)
                    osb = rot.tile([P, 512], BF16, tag="osb", name="osb",
                                   bufs=4)
                    if cast_scalar:
                        nc.scalar.copy(osb[:], opt_[:])
                        nc.scalar.dma_start(
                            out_r[oc * 128:(oc + 1) * 128, qs_o], osb[:])
                    else:
                        nc.vector.tensor_copy(osb[:], opt_[:])
                        nc.sync.dma_start(
                            out_r[oc * 128:(oc + 1) * 128, qs_o], osb[:])

                for qb in range(NQB):
                    qs = slice(qb * 512, (qb + 1) * 512)
                    for h in range(4):
                        p = h // 2
                        hv = (h % 2) * 128
                        ngr = 2 * (qb + 1)    # groups of 2 k-chunks
                        yps = ps3.tile([P, 512], F32, tag="ypv", name="ypv",
                                       bufs=1)
                        rsps = ps3.tile([P, 512], F32, tag="rs", name="rs",
                                        bufs=1)
                        for g in range(ngr):
                            scps = ps3.tile([P, 2, 512], F32, tag="sc",
                                            name="sc", bufs=2)
                            for j in range(2):
                                ki = 2 * g + j
                                _mm(nc, scps[:, j, :],
                                    kT[h][:, ki * 128:(ki + 1) * 128],
                                    qT[h][:, qs], True, True)
                            prb = rot.tile([P, 2, 512], BF16, tag="prb",
                                           name="prb", bufs=3)
                            nc.scalar.activation(
                                prb[:], scps[:], Exp, bias=ebias_sb[:],
                                scale=SCALE
                            )
                            for j in range(2):
                                ki = 2 * g + j
                                if g // 2 == qb:
                                    # diagonal chunk: only a 128-wide
                                    # triangle needs masking, and all
                                    # columns left of it are excluded
                                    # from the AV/rowsum matmuls instead
                                    # of being zeroed
                                    dd = ki - 4 * qb
                                    ds = slice(dd * 128, (dd + 1) * 128)
                                    nc.vector.tensor_mul(
                                        prb[:, j, ds], prb[:, j, ds],
                                        mask_sb[:, dd, ds]
                                    )
                                    cs = slice(dd * 128, 512)
                                else:
                                    cs = slice(0, 512)
                                first = ki == 0
                                last = ki == 2 * ngr - 1
                                _mm(nc, yps[:, cs],
                                    v_sb[p][:, ki, hv:hv + 128],
                                    prb[:, j, cs], first, last)
                                _mm(nc, rsps[:, cs], ones_sb[:],
                                    prb[:, j, cs], first, last)
                            # independent PE filler while exp runs
                            if pend_v and qb <= 1:
                                emit_v(cast_scalar=len(pend_v) % 2 == 0)
                                if pend_v:
                                    emit_v(cast_scalar=len(pend_v) % 2 == 0)
                            elif pend_o and not (h == 0 and g < 2) and \
                                    (qb < 3 or (h * ngr + g) % 2 == 0):
                                emit_o()
                        rcp = rot.tile([P, 512], F32, tag="rcp", name="rcp",
                                       bufs=2)
                        nc.vector.reciprocal_approx_fast(rcp[:], rsps[:])
                        nc.vector.tensor_mul(y_sb[h][:, qs], yps[:], rcp[:])
                    pend_o.extend((qb, oc) for oc in range(16))
                flip = False
                while pend_o:
                    emit_o(cast_scalar=flip)
                    flip = not flip

      if bench_io:
          with tc.tile_pool(name="bo", bufs=1) as bo:
              bt = bo.tile([1, 128], BF16, tag="bt", name="bt")
              nc.sync.dma_start(bt[:], out[0:1, 0:128])
              nc.sync.dma_start(outp[:, :], bt[:])

    nc.compile()
    return nc


_NC = None


def _get_nc():
    global _NC
    if _NC is None:
        _NC = build_program()
    return _NC


def _rope_tables():
    """cosA/sinB [128, S]: 32-row frequency pattern tiled 4x.
    sinB sign: rows [0:32] of each 64-block -> -sin, rows [32:64] -> +sin."""
    inv_freq = 1.0 / (BASE ** (np.arange(0, RD, 2, dtype=np.float32) / RD))  # [32]
    pos = np.arange(S, dtype=np.float32)
    ang = inv_freq[:, None] * pos[None, :]              # [32, S]
    cos1, sin1 = np.cos(ang), np.sin(ang)
    cosA = np.tile(cos1, (4, 1))                        # [128, S]
    sinB = np.concatenate([-sin1, sin1, -sin1, sin1], axis=0)
    return cosA.astype(_bf), sinB.astype(_bf)


def _mask_tiles():
    """masks[d][k, q] = 1.0 if q >= d*128 + k else 0 (bf16, [4,128,512])."""
    k = np.arange(P)[:, None]
    q = np.arange(512)[None, :]
    m = np.stack([(q >= d * 128 + k) for d in range(4)]).astype(np.float32)
    return np.ascontiguousarray(m.transpose(1, 0, 2)).astype(_bf)


def _prep_core_inputs(c, x, W_kv_d, W_q_d, W_k_u, W_q_u, W_v_u, W_rope_k, W_rope_q,
                      W_o, cosA, sinB, masks):
    b = c // 4
    hg = c % 4
    heads = [4 * hg + j for j in range(HEADS_PER_CORE)]

    def tile_pmaj(w):
        # [ko*128, m] -> [128, ko, m] partition-major for contiguous DMA
        ko = w.shape[0] // P
        return np.ascontiguousarray(
            w.reshape(ko, P, w.shape[1]).transpose(1, 0, 2))

    xT = np.ascontiguousarray(x[b].T).astype(_bf)                  # [H, S]
    w_kvd = tile_pmaj(np.ascontiguousarray(W_kv_d.T).astype(_bf))
    w_qd = tile_pmaj(np.ascontiguousarray(W_q_d.T).astype(_bf))

    # w_rk: per pair, rows [h1 rope dims | h0 rope dims], then transpose
    blocks = []
    for p in range(2):
        g0, g1 = heads[2 * p], heads[2 * p + 1]
        blocks.append(W_rope_k[g1 * RD:(g1 + 1) * RD, :])
        blocks.append(W_rope_k[g0 * RD:(g0 + 1) * RD, :])
    w_rk = tile_pmaj(np.ascontiguousarray(np.concatenate(blocks, axis=0).T).astype(_bf))

    # w_qc: per local head 128 cols: even -> [content|rope], odd -> [rope|content]
    cols = []
    for j, g in enumerate(heads):
        c_blk = W_q_u[g * RD:(g + 1) * RD, :].T       # [LAT, 64]
        r_blk = W_rope_q[g * RD:(g + 1) * RD, :].T    # [LAT, 64]
        cols.extend([c_blk, r_blk] if j % 2 == 0 else [r_blk, c_blk])
    w_qc = tile_pmaj(np.ascontiguousarray(np.concatenate(cols, axis=1)).astype(_bf))

    # w_kc: per pair 128 cols: [h0 content | h1 content]
    cols = []
    for p in range(2):
        g0, g1 = heads[2 * p], heads[2 * p + 1]
        cols.append(W_k_u[g0 * RD:(g0 + 1) * RD, :].T)
        cols.append(W_k_u[g1 * RD:(g1 + 1) * RD, :].T)
    w_kc = tile_pmaj(np.ascontiguousarray(np.concatenate(cols, axis=1)).astype(_bf))

    # w_v: per pair 256 cols: [h0 v dims | h1 v dims]
    cols = []
    for p in range(2):
        g0, g1 = heads[2 * p], heads[2 * p + 1]
        cols.append(W_v_u[g0 * HD:(g0 + 1) * HD, :].T)
        cols.append(W_v_u[g1 * HD:(g1 + 1) * HD, :].T)
    w_v = tile_pmaj(np.ascontiguousarray(np.concatenate(cols, axis=1)).astype(_bf))

    d0 = heads[0] * HD
    w_o = tile_pmaj(np.ascontiguousarray(W_o[:, d0:d0 + 512].T).astype(_bf))

    return {
        "xT": xT, "w_kvd": w_kvd, "w_qd": w_qd, "w_rk": w_rk, "w_qc": w_qc,
        "w_kc": w_kc, "w_v": w_v, "w_o": w_o, "cosA": cosA, "sinB": sinB,
        "masks": masks,
    }


def make_in_maps(inputs):
    x = np.asarray(inputs["hidden_states"], dtype=np.float32)
    ws = {k: np.asarray(inputs[k], dtype=np.float32)
          for k in ("W_kv_d", "W_q_d", "W_k_u", "W_q_u", "W_v_u", "W_rope_k",
                    "W_rope_q", "W_o")}
    cosA, sinB = _rope_tables()
    masks = _mask_tiles()
    return [
        _prep_core_inputs(c, x, ws["W_kv_d"], ws["W_q_d"], ws["W_k_u"],
                          ws["W_q_u"], ws["W_v_u"], ws["W_rope_k"],
                          ws["W_rope_q"], ws["W_o"], cosA, sinB, masks)
        for c in range(N_CORES)
    ]


def assemble(results):
    """results: list of 8 dicts with 'out' [H, S] f32 partials (transposed)."""
    full = np.empty((B, S, H), dtype=np.float32)
    for b in range(B):
        acc = results[4 * b]["out"].astype(np.float32)
        for g in range(1, 4):
            acc = acc + results[4 * b + g]["out"]
        full[b] = acc.T
    return full


def kernel(**inputs):
    nc = _get_nc()
    in_maps = make_in_maps(inputs)
    res = run_bass_kernel_spmd(nc, in_maps, core_ids=list(range(N_CORES)))
    return assemble(res.results)


# revision 51
# speedup vs baseline: 1.0443x; 1.0230x over previous
"""MLA (multi-head latent attention) Trainium2 kernel.

Sharding: 8 cores = 2 (batch) x 4 (head groups of 4 heads).
Each core computes, for its batch b and heads [4g, 4g+4):
  latents kv_d/q_d (replicated within the batch group), per-head
  up-projections + RoPE, causal SDPA, and a partial o_proj
  out_core[o, q] = sum_{d in core's 512 head-dims} W_o[o, d] * y[d, q].
Host sums the 4 partials per batch (the all-reduce step of the hint,
performed at unshard time) and transposes to [S, H].

All matmuls run in bf16 with fp32 PSUM accumulation.

Structure:
- Phase 1 streams xT ONCE (16 k-chunks x 4 S-quarters, chunk-major)
  and computes kv_d, q_d and raw rope-k together; weight pieces are
  loaded in need-order across two DMA queues so the PE starts early.
- PSUM->SBUF copies are spread across Scalar and Vector; rope swap
  transposes issue on the idle gpsimd DMA queue.
- SDPA: for diagonal (causal-boundary) key chunks only a 128-wide
  triangle is masked and all columns left of it are excluded from the
  AV/rowsum matmuls (narrower free dim) instead of being zeroed.
- o_proj (lagged by one q-block, paced every other group in the last
  block) and the tail of the v up-projection are interleaved into the
  SDPA group loop so the PE has independent work while the scalar
  engine computes exp().
"""

import numpy as np
import ml_dtypes

import concourse.bass as bass
import concourse.mybir as mybir
import concourse.tile as tile
from concourse import bacc
from concourse._compat import get_trn_type
from concourse.bass_utils import run_bass_kernel_spmd

H = 2048
NH = 16
HD = 128           # head dim
RD = 64            # rotary dim
RH = 32            # rotary half
LAT = 256
B = 2
S = 2048
BASE = 10000.0
N_CORES = 8
HEADS_PER_CORE = 4
P = 128
NQB = S // 512     # 4 query blocks of 512
NKI = S // 128     # 16 key chunks of 128
SCALE = 1.0 / np.sqrt(float(HD))
EXP_BIAS = -4.0

BF16 = mybir.dt.bfloat16
F32 = mybir.dt.float32
_bf = ml_dtypes.bfloat16


def _mm(nc, out, lhsT, rhs, start, stop):
    nc.tensor.matmul(out, lhsT, rhs, start=start, stop=stop)


def build_program(nrep=1, bench_io=False):
    nc = bacc.Bacc(
        get_trn_type() or "TRN2",
        target_bir_lowering=False,
        debug=False,
        num_devices=N_CORES,
    )

    if bench_io:
        dummy = nc.declare_dram_parameter("bdummy", [1, 128], F32, isOutput=False)
        xT = nc.dram_tensor("xT", [H, S], BF16)
        w_kvd = nc.dram_tensor("w_kvd", [P, 16, LAT], BF16)
        w_qd = nc.dram_tensor("w_qd", [P, 16, LAT], BF16)
        w_rk = nc.dram_tensor("w_rk", [P, 16, 256], BF16)
        w_qc = nc.dram_tensor("w_qc", [P, 2, 512], BF16)
        w_kc = nc.dram_tensor("w_kc", [P, 2, 256], BF16)
        w_v = nc.dram_tensor("w_v", [P, 2, 512], BF16)
        w_o = nc.dram_tensor("w_o", [P, 4, H], BF16)
        cosA = nc.dram_tensor("cosA", [P, S], BF16)
        sinB = nc.dram_tensor("sinB", [P, S], BF16)
        masks = nc.dram_tensor("masks", [P, 4, 512], BF16)
        out = nc.dram_tensor("outs", [H, S], BF16)
        outp = nc.declare_dram_parameter("out", [1, 128], BF16, isOutput=True)
    else:
        xT = nc.declare_dram_parameter("xT", [H, S], BF16, isOutput=False)
        w_kvd = nc.declare_dram_parameter("w_kvd", [P, 16, LAT], BF16, isOutput=False)
        w_qd = nc.declare_dram_parameter("w_qd", [P, 16, LAT], BF16, isOutput=False)
        w_rk = nc.declare_dram_parameter("w_rk", [P, 16, 256], BF16, isOutput=False)
        w_qc = nc.declare_dram_parameter("w_qc", [P, 2, 512], BF16, isOutput=False)
        w_kc = nc.declare_dram_parameter("w_kc", [P, 2, 256], BF16, isOutput=False)
        w_v = nc.declare_dram_parameter("w_v", [P, 2, 512], BF16, isOutput=False)
        w_o = nc.declare_dram_parameter("w_o", [P, 4, H], BF16, isOutput=False)
        cosA = nc.declare_dram_parameter("cosA", [P, S], BF16, isOutput=False)
        sinB = nc.declare_dram_parameter("sinB", [P, S], BF16, isOutput=False)
        masks = nc.declare_dram_parameter("masks", [P, 4, 512], BF16, isOutput=False)
        out = nc.declare_dram_parameter("out", [H, S], BF16, isOutput=True)

    Exp = mybir.ActivationFunctionType.Exp

    scratch = (nc.dram_tensor("scratch", [H, S], BF16) if nrep > 1 else None)

    with tile.TileContext(nc) as tc:
      for rep in range(nrep):
        out_r = out if rep == nrep - 1 else scratch
        with (
            tc.tile_pool(name=f"wpool{rep}", bufs=1) as wpool,
            tc.tile_pool(name=f"main1_{rep}", bufs=1) as main1,
            tc.tile_pool(name=f"main2_{rep}", bufs=1) as main2,
            tc.tile_pool(name=f"ph1_{rep}", bufs=1) as ph1,
            tc.tile_pool(name=f"rot{rep}", bufs=3) as rot,
        ):
            # -------- persistent small tensors (DMAs deferred) --------
            cos_sb = wpool.tile([P, S], BF16, tag="cos", name="cos")
            sin_sb = wpool.tile([P, S], BF16, tag="sin", name="sin")
            mask_sb = wpool.tile([P, 4, 512], BF16, tag="mask", name="mask")
            ones_sb = wpool.tile([P, P], BF16, tag="ones", name="ones")
            nc.gpsimd.memset(ones_sb[:], 1.0)
            ebias_sb = wpool.tile([P, 1], F32, tag="ebias", name="ebias")
            nc.gpsimd.memset(ebias_sb[:], EXP_BIAS)
            wo_sb = wpool.tile([P, 4, H], BF16, tag="wo", name="wo")
            wqc_sb = wpool.tile([P, 2, 512], BF16, tag="wqc", name="wqc")
            wkc_sb = wpool.tile([P, 2, 256], BF16, tag="wkc", name="wkc")
            wv_sb = wpool.tile([P, 2, 512], BF16, tag="wv", name="wv")

            # -------- phase-1 outputs (latents), bf16 --------
            kvd_sb = [main1.tile([P, S], BF16, tag=f"kvd{m}", name=f"kvd{m}")
                      for m in range(2)]
            qd_sb = [main1.tile([P, S], BF16, tag=f"qd{m}", name=f"qd{m}")
                     for m in range(2)]

            # phase-1 weights, loaded in need-order pieces interleaved with
            # the first x chunks: chunk-major matmuls need chunk k of all
            # three weight tensors, so stream them k-range by k-range.
            wkvd_sb = ph1.tile([P, 16, LAT], BF16, tag="wkvd", name="wkvd")
            wqd_sb = ph1.tile([P, 16, LAT], BF16, tag="wqd", name="wqd")
            wrk_sb = ph1.tile([P, 16, 256], BF16, tag="wrk", name="wrk")
            # latent weights stream on the scalar queue in parallel with
            # the sync queue's wrk + x chunks for faster early ramp; only
            # the first wrk piece goes ahead of the x chunks
            nc.sync.dma_start(wrk_sb[:, 0:1, :], w_rk[:, 0:1, :])
            for ks in (slice(0, 1), slice(1, 2), slice(2, 4)):
                nc.scalar.dma_start(wkvd_sb[:, ks, :], w_kvd[:, ks, :])
                nc.scalar.dma_start(wqd_sb[:, ks, :], w_qd[:, ks, :])

            # -------- per-head q/k tiles (dims on partitions), v, y --------
            # even head h: rows [0:64] content, [64:128] rope
            # odd  head h: rows [0:64] rope,    [64:128] content
            qT = [main2.tile([P, S], BF16, tag=f"qT{h}", name=f"qT{h}")
                  for h in range(4)]
            kT = [main2.tile([P, S], BF16, tag=f"kT{h}", name=f"kT{h}")
                  for h in range(4)]
            v_sb = [main2.tile([P, NKI, 256], BF16, tag=f"v{p}", name=f"v{p}")
                    for p in range(2)]
            y_sb = [main2.tile([P, S], BF16, tag=f"y{h}", name=f"y{h}")
                    for h in range(4)]

            # -------- phase 1: single pass over x, S in 4 quarters --------
            # Per quarter: 6 PSUM banks accumulate (kr0, kr1, kvd0, kvd1,
            # qd0, qd1) over the 16 k-chunks, chunk-major so the PE starts
            # as soon as the first weight pieces + x chunk land.
            with tc.tile_pool(name=f"ps1_{rep}", bufs=1, space="PSUM") as ps1:
                for n in range(4):
                    qs = slice(n * 512, (n + 1) * 512)
                    xq = ph1.tile([P, NKI, 512], BF16, tag="xq", name="xq",
                                  bufs=2)
                    for k in range(NKI):
                        nc.sync.dma_start(xq[:, k, :],
                                          xT[k * 128:(k + 1) * 128, qs])
                        if n == 0:
                            # remaining wrk pieces interleave with the x
                            # chunks (sync head, gpsimd tail) and the
                            # latent-weight tails ride the scalar queue,
                            # so the x stream keeps its DMA-engine share
                            if k == 0:
                                nc.sync.dma_start(wrk_sb[:, 1:2, :],
                                                  w_rk[:, 1:2, :])
                            elif k == 1:
                                nc.sync.dma_start(wrk_sb[:, 2:4, :],
                                                  w_rk[:, 2:4, :])
                                nc.gpsimd.dma_start(wrk_sb[:, 4:8, :],
                                                    w_rk[:, 4:8, :])
                                nc.scalar.dma_start(wkvd_sb[:, 4:8, :],
                                                    w_kvd[:, 4:8, :])
                                nc.scalar.dma_start(wqd_sb[:, 4:8, :],
                                                    w_qd[:, 4:8, :])
                            elif k == 3:
                                nc.gpsimd.dma_start(wrk_sb[:, 8:16, :],
                                                    w_rk[:, 8:16, :])
                                nc.scalar.dma_start(wkvd_sb[:, 8:16, :],
                                                    w_kvd[:, 8:16, :])
                                nc.scalar.dma_start(wqd_sb[:, 8:16, :],
                                                    w_qd[:, 8:16, :])
                    if n == 0:
                        nc.sync.dma_start(cos_sb[:], cosA[:, :])
                        nc.sync.dma_start(sin_sb[:], sinB[:, :])
                    elif n == 1:
                        nc.sync.dma_start(mask_sb[:], masks[:, :, :])
                        nc.sync.dma_start(wqc_sb[:], w_qc[:, :, :])
                        nc.sync.dma_start(wkc_sb[:], w_kc[:, :, :])
                        nc.sync.dma_start(wv_sb[:], w_v[:, :, :])
                    elif n == 2:
                        nc.sync.dma_start(wo_sb[:], w_o[:, :, :])

                    krb = [ps1.tile([P, 512], F32, tag=f"kr{p}",
                                    name=f"kr{p}", bufs=2) for p in range(2)]
                    lat = [ps1.tile([P, 512], F32, tag=f"lat{i}",
                                    name=f"lat{i}") for i in range(4)]
                    plan = [
                        (wrk_sb, 0, krb[0]),
                        (wrk_sb, 1, krb[1]),
                        (wkvd_sb, 0, lat[0]),
                        (wkvd_sb, 1, lat[1]),
                        (wqd_sb, 0, lat[2]),
                        (wqd_sb, 1, lat[3]),
                    ]
                    for k in range(NKI):
                        for wsb, mi, bank in plan:
                            _mm(nc, bank[:],
                                wsb[:, k, mi * 128:(mi + 1) * 128],
                                xq[:, k, :], k == 0, k == NKI - 1)

                    # latent copies first (their banks gate the next
                    # quarter; kraw banks are double-buffered); split
                    # across Scalar and Vector to halve the drain
                    targets = [kvd_sb[0], kvd_sb[1], qd_sb[0], qd_sb[1]]
                    for i, tgt in enumerate(targets):
                        if i < 2:
                            nc.scalar.copy(tgt[:, qs], lat[i][:])
                        else:
                            nc.vector.tensor_copy(tgt[:, qs], lat[i][:])

                    # rope on raw k for this quarter:
                    # krb[p] rows [0:64]=h1 rope, [64:128]=h0 rope
                    for p in range(2):
                        h0, h1 = 2 * p, 2 * p + 1
                        kraw = rot.tile([P, 512], BF16, tag="krawc",
                                        name="krawc", bufs=3)
                        if n == 3:
                            # last quarter: keep Scalar free for the
                            # phase-2 copies that gate the next matmuls
                            nc.vector.tensor_copy(kraw[:], krb[p][:])
                        else:
                            nc.scalar.copy(kraw[:], krb[p][:])
                        swp = rot.tile([P, 512], BF16, tag="swp", name="swp",
                                       bufs=3)
                        nc.gpsimd.dma_start(swp[0:32, :], kraw[32:64, :])
                        nc.gpsimd.dma_start(swp[32:64, :], kraw[0:32, :])
                        nc.gpsimd.dma_start(swp[64:96, :], kraw[96:128, :])
                        nc.gpsimd.dma_start(swp[96:128, :], kraw[64:96, :])
                        nc.vector.tensor_mul(swp[:], swp[:], sin_sb[:, qs])
                        nc.vector.tensor_mul(
                            kT[h1][0:64, qs], kraw[0:64, :], cos_sb[0:64, qs]
                        )
                        nc.vector.tensor_add(
                            kT[h1][0:64, qs], kT[h1][0:64, qs], swp[0:64, :]
                        )
                        nc.vector.tensor_mul(
                            kT[h0][64:128, qs], kraw[64:128, :],
                            cos_sb[64:128, qs]
                        )
                        nc.vector.tensor_add(
                            kT[h0][64:128, qs], kT[h0][64:128, qs],
                            swp[64:128, :]
                        )

            # -------- phase 2: q up-proj + rope (head-major, so the DVE
            # rope chain for head h overlaps later heads' matmuls and the
            # first SDPA blocks), k content (copies on Pool), v head --------
            with tc.tile_pool(name=f"ps2_{rep}", bufs=2, space="PSUM") as ps2:
                def qcomb_head(h):
                    for n2 in range(4):
                        ns = slice(n2 * 512, (n2 + 1) * 512)
                        pt = ps2.tile([P, 512], F32, tag="qcomb",
                                      name="qcomb", bufs=3)
                        for kc in range(2):
                            _mm(nc, pt[:],
                                wqc_sb[:, kc, h * 128:(h + 1) * 128],
                                qd_sb[kc][:, ns], kc == 0, kc == 1)
                        nc.scalar.copy(qT[h][:, ns], pt[:])
                    r = 64 if h % 2 == 0 else 0
                    swp = rot.tile([P, S], BF16, tag="swpq", name="swpq",
                                   bufs=2)
                    nc.sync.dma_start(swp[r:r + 32, :],
                                      qT[h][r + 32:r + 64, :])
                    nc.sync.dma_start(swp[r + 32:r + 64, :],
                                      qT[h][r:r + 32, :])
                    nc.vector.tensor_mul(
                        qT[h][r:r + 64, :], qT[h][r:r + 64, :],
                        cos_sb[r:r + 64, :]
                    )
                    nc.vector.tensor_mul(
                        swp[r:r + 64, :], swp[r:r + 64, :], sin_sb[r:r + 64, :]
                    )
                    nc.vector.tensor_add(
                        qT[h][r:r + 64, :], qT[h][r:r + 64, :], swp[r:r + 64, :]
                    )

                def kc_pair(p2):
                    # k content: psum rows [0:64]=h0, [64:128]=h1
                    h0, h1 = 2 * p2, 2 * p2 + 1
                    for n2 in range(4):
                        ns = slice(n2 * 512, (n2 + 1) * 512)
                        pt = ps2.tile([P, 512], F32, tag="kcont",
                                      name="kcont")
                        for kc in range(2):
                            _mm(nc, pt[:],
                                wkc_sb[:, kc, p2 * 128:(p2 + 1) * 128],
                                kvd_sb[kc][:, ns], kc == 0, kc == 1)
                        nc.scalar.copy(kT[h0][0:64, ns], pt[0:64, :])
                        nc.vector.tensor_copy(kT[h1][64:128, ns],
                                              pt[64:128, :])

                def v_first(p2):
                    # v for the first 8 key chunks (the rest is interleaved
                    # into the SDPA loop as PE filler work); phase 2 has
                    # spare PE time, so front-load the matmuls here
                    for s16 in range(8):
                        pt = ps2.tile([P, 256], F32, tag="vps", name="vps")
                        for kc in range(2):
                            _mm(nc, pt[:],
                                kvd_sb[kc][:, s16 * 128:(s16 + 1) * 128],
                                wv_sb[:, kc, p2 * 256:(p2 + 1) * 256],
                                kc == 0, kc == 1)
                        if s16 % 2 == 0:
                            nc.scalar.copy(v_sb[p2][:, s16, :], pt[:])
                        else:
                            nc.vector.tensor_copy(v_sb[p2][:, s16, :], pt[:])

                qcomb_head(0)
                kc_pair(0)
                qcomb_head(1)
                v_first(0)
                kc_pair(1)
                qcomb_head(2)
                v_first(1)
                qcomb_head(3)

            # -------- SDPA + interleaved o_proj / v tail --------
            with tc.tile_pool(name=f"ps3_{rep}", bufs=1, space="PSUM") as ps3:
                pend_v = [(s16, p) for s16 in range(8, NKI) for p in range(2)]
                pend_o = []

                def emit_v(cast_scalar=False):
                    s16, p = pend_v.pop(0)
                    pt = ps3.tile([P, 512], F32, tag="aux", name="vaux",
                                  bufs=2)
                    for kc in range(2):
                        _mm(nc, pt[:, 0:256],
                            kvd_sb[kc][:, s16 * 128:(s16 + 1) * 128],
                            wv_sb[:, kc, p * 256:(p + 1) * 256],
                            kc == 0, kc == 1)
                    if cast_scalar:
                        nc.scalar.copy(v_sb[p][:, s16, :], pt[:, 0:256])
                    else:
                        nc.vector.tensor_copy(v_sb[p][:, s16, :],
                                              pt[:, 0:256])

                def emit_o(cast_scalar=False):
                    qb_o, oc = pend_o.pop(0)
                    qs_o = slice(qb_o * 512, (qb_o + 1) * 512)
                    opt_ = ps3.tile([P, 512], F32, tag="aux", name="opj",
                                    bufs=2)
                    for hk in range(4):
                        _mm(nc, opt_[:],
                            wo_sb[:, hk, oc * 128:(oc + 1) * 128],
                            y_sb[hk][:, qs_o], hk == 0, hk == 3)
                    osb = rot.tile([P, 512], BF16, tag="osb", name="osb",
                                   bufs=4)
                    if cast_scalar:
                        nc.scalar.copy(osb[:], opt_[:])
                        nc.scalar.dma_start(
                            out_r[oc * 128:(oc + 1) * 128, qs_o], osb[:])
                    else:
                        nc.vector.tensor_copy(osb[:], opt_[:])
                        nc.sync.dma_start(
                            out_r[oc * 128:(oc + 1) * 128, qs_o], osb[:])

                for qb in range(NQB):
                    qs = slice(qb * 512, (qb + 1) * 512)
                    for h in range(4):
                        p = h // 2
                        hv = (h % 2) * 128
                        ngr = 2 * (qb + 1)    # groups of 2 k-chunks
                        yps = ps3.tile([P, 512], F32, tag="ypv", name="ypv",
                                       bufs=1)
                        rsps = ps3.tile([P, 512], F32, tag="rs", name="rs",
                                        bufs=1)
                        for g in range(ngr):
                            diag = g // 2 == qb
                            scps = ps3.tile([P, 2, 512], F32, tag="sc",
                                            name="sc", bufs=2)
                            for j in range(2):
                                ki = 2 * g + j
                                # diagonal chunk ki: columns left of the
                                # causal boundary are dead — narrow the
                                # scores matmul to [dd*128, 512)
                                c0 = (ki - 4 * qb) * 128 if diag else 0
                                _mm(nc, scps[:, j, c0:512],
                                    kT[h][:, ki * 128:(ki + 1) * 128],
                                    qT[h][:, qb * 512 + c0:(qb + 1) * 512],
                                    True, True)
                            prb = rot.tile([P, 2, 512], BF16, tag="prb",
                                           name="prb", bufs=3)
                            if diag:
                                for j in range(2):
                                    c0 = (2 * g + j - 4 * qb) * 128
                                    nc.scalar.activation(
                                        prb[:, j, c0:512],
                                        scps[:, j, c0:512], Exp,
                                        bias=ebias_sb[:], scale=SCALE
                                    )
                            else:
                                nc.scalar.activation(
                                    prb[:], scps[:], Exp, bias=ebias_sb[:],
                                    scale=SCALE
                                )
                            for j in range(2):
                                ki = 2 * g + j
                                if diag:
                                    # only a 128-wide triangle needs
                                    # masking; the columns left of it are
                                    # excluded from the AV/rowsum matmuls
                                    dd = ki - 4 * qb
                                    ds = slice(dd * 128, (dd + 1) * 128)
                                    nc.vector.tensor_mul(
                                        prb[:, j, ds], prb[:, j, ds],
                                        mask_sb[:, dd, ds]
                                    )
                                    cs = slice(dd * 128, 512)
                                else:
                                    cs = slice(0, 512)
                                first = ki == 0
                                last = ki == 2 * ngr - 1
                                _mm(nc, yps[:, cs],
                                    v_sb[p][:, ki, hv:hv + 128],
                                    prb[:, j, cs], first, last)
                                _mm(nc, rsps[:, cs], ones_sb[:],
                                    prb[:, j, cs], first, last)
                            # independent PE filler while exp runs
                            if pend_v and qb <= 1:
                                emit_v(cast_scalar=len(pend_v) % 2 == 0)
                                if pend_v:
                                    emit_v(cast_scalar=len(pend_v) % 2 == 0)
                            elif pend_o and not (h == 0 and g < 2) and \
                                    (qb < 3 or (h * ngr + g) % 2 == 0):
                                emit_o()
                        rcp = rot.tile([P, 512], F32, tag="rcp", name="rcp",
                                       bufs=2)
                        nc.vector.reciprocal_approx_fast(rcp[:], rsps[:])
                        nc.vector.tensor_mul(y_sb[h][:, qs], yps[:], rcp[:])
                    pend_o.extend((qb, oc) for oc in range(16))
                flip = False
                while pend_o:
                    emit_o(cast_scalar=flip)
                    flip = not flip

      if bench_io:
          with tc.tile_pool(name="bo", bufs=1) as bo:
              bt = bo.tile([1, 128], BF16, tag="bt", name="bt")
              nc.sync.dma_start(bt[:], out[0:1, 0:128])
              nc.sync.dma_start(outp[:, :], bt[:])

    nc.compile()
    return nc


_NC = None


def _get_nc():
    global _NC
    if _NC is None:
        _NC = build_program()
    return _NC


def _rope_tables():
    """cosA/sinB [128, S]: 32-row frequency pattern tiled 4x.
    sinB sign: rows [0:32] of each 64-block -> -sin, rows [32:64] -> +sin."""
    inv_freq = 1.0 / (BASE ** (np.arange(0, RD, 2, dtype=np.float32) / RD))  # [32]
    pos = np.arange(S, dtype=np.float32)
    ang = inv_freq[:, None] * pos[None, :]              # [32, S]
    cos1, sin1 = np.cos(ang), np.sin(ang)
    cosA = np.tile(cos1, (4, 1))                        # [128, S]
    sinB = np.concatenate([-sin1, sin1, -sin1, sin1], axis=0)
    return cosA.astype(_bf), sinB.astype(_bf)


def _mask_tiles():
    """masks[d][k, q] = 1.0 if q >= d*128 + k else 0 (bf16, [4,128,512])."""
    k = np.arange(P)[:, None]
    q = np.arange(512)[None, :]
    m = np.stack([(q >= d * 128 + k) for d in range(4)]).astype(np.float32)
    return np.ascontiguousarray(m.transpose(1, 0, 2)).astype(_bf)


def _prep_core_inputs(c, x, W_kv_d, W_q_d, W_k_u, W_q_u, W_v_u, W_rope_k, W_rope_q,
                      W_o, cosA, sinB, masks):
    b = c // 4
    hg = c % 4
    heads = [4 * hg + j for j in range(HEADS_PER_CORE)]

    def tile_pmaj(w):
        # [ko*128, m] -> [128, ko, m] partition-major for contiguous DMA
        ko = w.shape[0] // P
        return np.ascontiguousarray(
            w.reshape(ko, P, w.shape[1]).transpose(1, 0, 2))

    xT = np.ascontiguousarray(x[b].T).astype(_bf)                  # [H, S]
    w_kvd = tile_pmaj(np.ascontiguousarray(W_kv_d.T).astype(_bf))
    w_qd = tile_pmaj(np.ascontiguousarray(W_q_d.T).astype(_bf))

    # w_rk: per pair, rows [h1 rope dims | h0 rope dims], then transpose
    blocks = []
    for p in range(2):
        g0, g1 = heads[2 * p], heads[2 * p + 1]
        blocks.append(W_rope_k[g1 * RD:(g1 + 1) * RD, :])
        blocks.append(W_rope_k[g0 * RD:(g0 + 1) * RD, :])
    w_rk = tile_pmaj(np.ascontiguousarray(np.concatenate(blocks, axis=0).T).astype(_bf))

    # w_qc: per local head 128 cols: even -> [content|rope], odd -> [rope|content]
    cols = []
    for j, g in enumerate(heads):
        c_blk = W_q_u[g * RD:(g + 1) * RD, :].T       # [LAT, 64]
        r_blk = W_rope_q[g * RD:(g + 1) * RD, :].T    # [LAT, 64]
        cols.extend([c_blk, r_blk] if j % 2 == 0 else [r_blk, c_blk])
    w_qc = tile_pmaj(np.ascontiguousarray(np.concatenate(cols, axis=1)).astype(_bf))

    # w_kc: per pair 128 cols: [h0 content | h1 content]
    cols = []
    for p in range(2):
        g0, g1 = heads[2 * p], heads[2 * p + 1]
        cols.append(W_k_u[g0 * RD:(g0 + 1) * RD, :].T)
        cols.append(W_k_u[g1 * RD:(g1 + 1) * RD, :].T)
    w_kc = tile_pmaj(np.ascontiguousarray(np.concatenate(cols, axis=1)).astype(_bf))

    # w_v: per pair 256 cols: [h0 v dims | h1 v dims]
    cols = []
    for p in range(2):
        g0, g1 = heads[2 * p], heads[2 * p + 1]
        cols.append(W_v_u[g0 * HD:(g0 + 1) * HD, :].T)
        cols.append(W_v_u[g1 * HD:(g1 + 1) * HD, :].T)
    w_v = tile_pmaj(np.ascontiguousarray(np.concatenate(cols, axis=1)).astype(_bf))

    d0 = heads[0] * HD
    w_o = tile_pmaj(np.ascontiguousarray(W_o[:, d0:d0 + 512].T).astype(_bf))

    return {
        "xT": xT, "w_kvd": w_kvd, "w_qd": w_qd, "w_rk": w_rk, "w_qc": w_qc,
        "w_kc": w_kc, "w_v": w_v, "w_o": w_o, "cosA": cosA, "sinB": sinB,
        "masks": masks,
    }


def make_in_maps(inputs):
    x = np.asarray(inputs["hidden_states"], dtype=np.float32)
    ws = {k: np.asarray(inputs[k], dtype=np.float32)
          for k in ("W_kv_d", "W_q_d", "W_k_u", "W_q_u", "W_v_u", "W_rope_k",
                    "W_rope_q", "W_o")}
    cosA, sinB = _rope_tables()
    masks = _mask_tiles()
    return [
        _prep_core_inputs(c, x, ws["W_kv_d"], ws["W_q_d"], ws["W_k_u"],
                          ws["W_q_u"], ws["W_v_u"], ws["W_rope_k"],
                          ws["W_rope_q"], ws["W_o"], cosA, sinB, masks)
        for c in range(N_CORES)
    ]


def assemble(results):
    """results: list of 8 dicts with 'out' [H, S] f32 partials (transposed)."""
    full = np.empty((B, S, H), dtype=np.float32)
    for b in range(B):
        acc = results[4 * b]["out"].astype(np.float32)
        for g in range(1, 4):
            acc = acc + results[4 * b + g]["out"]
        full[b] = acc.T
    return full


def kernel(**inputs):
    nc = _get_nc()
    in_maps = make_in_maps(inputs)
    res = run_bass_kernel_spmd(nc, in_maps, core_ids=list(range(N_CORES)))
    return assemble(res.results)
